# revision 12
# baseline (speedup 1.0000x reference)
"""Trainium2 Bass kernel for nn_DTIModel (EGNN message passing + DTI heads).

Sharding: data-parallel over graphs. Core k owns nodes [k*8192, (k+1)*8192)
(= graphs [k*256, (k+1)*256)) and all edges whose destination (row) lands in
that range, so both segment-sums stay local. h/pos live in a combined
[N, 132] bf16 gather table that is AllGather'd after each layer's node
update. The f32 residual stream for local nodes stays in SBUF.

Self-contained: hardcodes all shapes; host-side prep is plain numpy.
"""

import math
from dataclasses import dataclass, field

import numpy as np
import ml_dtypes

import concourse.bass as bass
import concourse.mybir as mybir
import concourse.tile as tile
from concourse import bacc
from concourse.bass import IndirectOffsetOnAxis
from concourse.bass_utils import run_bass_kernel_spmd
from concourse.masks import make_identity

F32 = mybir.dt.float32
BF16 = mybir.dt.bfloat16
I32 = mybir.dt.int32
AF = mybir.ActivationFunctionType
ALU = mybir.AluOpType

EPS = 1e-8


@dataclass
class Cfg:
    ncores: int = 8
    nloc: int = 8192          # nodes per core
    layers: int = 4
    ep: int = 640             # padded edge slots per 128-node block
    nd: int = 32
    ed: int = 8
    h: int = 128
    pe: int = 1280
    ph: int = 512
    ch: int = 512
    graph: int = 32           # nodes per graph

    @property
    def blocks(self):
        return self.nloc // 128

    @property
    def cpb(self):
        return self.ep // 128  # chunks per block

    @property
    def slots(self):
        return self.blocks * self.ep  # padded edge slots per core

    @property
    def chunks(self):
        return self.slots // 128

    @property
    def n(self):
        return self.ncores * self.nloc

    @property
    def gpc(self):
        return self.nloc // self.graph  # graphs per core

    @property
    def tw(self):
        return self.h + 4  # table width (bf16): h + pos(3)+pad


# ---------------------------------------------------------------- host prep


def _to_bf(x):
    return np.asarray(x, np.float32).astype(ml_dtypes.bfloat16)


def prep_inputs(cfg: Cfg, inputs: dict):
    """Split/reorder inputs per core. Returns (in_maps, EP-actualized cfg)."""
    N, H, ED = cfg.n, cfg.h, cfg.ed
    x = np.asarray(inputs["x"], np.float32)
    pos = np.asarray(inputs["pos"], np.float32)
    edge_attr = np.asarray(inputs["edge_attr"], np.float32)
    prot = np.asarray(inputs["protein_embedding"], np.float32)
    edge_index = np.asarray(inputs["edge_index"], np.int64)
    row = edge_index[0].astype(np.int64)
    col = edge_index[1].astype(np.int64)

    h0 = x @ np.asarray(inputs["emb_in_w"], np.float32) + np.asarray(
        inputs["emb_in_b"], np.float32
    )

    # per-node edge counts (for seg_mean of coord updates)
    cnt = np.bincount(row, minlength=N).astype(np.float32)
    invc_full = 1.0 / np.maximum(cnt, 1.0)

    # order edges by row; per-core, per 128-node block padded segments
    order = np.argsort(row, kind="stable")
    row_s = row[order]
    col_s = col[order]
    attr_s = edge_attr[order]

    blk_of_edge = row_s // 128  # global block id, sorted ascending
    nblocks_g = N // 128
    blk_counts = np.bincount(blk_of_edge, minlength=nblocks_g)
    cfg.ep = int(((blk_counts.max() + 127) // 128) * 128)
    blk_starts = np.zeros(nblocks_g + 1, np.int64)
    np.cumsum(blk_counts, out=blk_starts[1:])

    in_maps = []
    shared = {}

    # ---- shared weights
    w1 = np.asarray(inputs["edge_w1"], np.float32)
    for l in range(cfg.layers):
        shared[f"w1a_{l}"] = _to_bf(w1[l, 0:H])
        shared[f"w1b_{l}"] = _to_bf(w1[l, H : 2 * H])
        # rows: [radial(1); attr(8)]
        w1c = np.concatenate([w1[l, 2 * H : 2 * H + 1], w1[l, 2 * H + 1 :]], 0)
        shared[f"w1c_{l}"] = _to_bf(w1c)
        shared[f"w2_{l}"] = _to_bf(inputs["edge_w2"][l])
        shared[f"attw_{l}"] = _to_bf(inputs["att_w"][l])
        shared[f"cw1_{l}"] = _to_bf(inputs["coord_w1"][l])
        shared[f"cw2_{l}"] = _to_bf(inputs["coord_w2"][l])
        nw1 = np.asarray(inputs["node_w1"], np.float32)
        shared[f"nw1h_{l}"] = _to_bf(nw1[l, 0:H])
        shared[f"nw1a_{l}"] = _to_bf(nw1[l, H : 2 * H])
        shared[f"nw2_{l}"] = _to_bf(inputs["node_w2"][l])
        shared[f"b1_{l}"] = np.asarray(inputs["edge_b1"][l], np.float32).reshape(H, 1)
        shared[f"b2_{l}"] = np.asarray(inputs["edge_b2"][l], np.float32).reshape(H, 1)
        shared[f"attb_{l}"] = np.asarray(inputs["att_b"][l], np.float32).reshape(1, 1)
        shared[f"cb1_{l}"] = np.asarray(inputs["coord_b1"][l], np.float32).reshape(H, 1)
        shared[f"nb1_{l}"] = np.asarray(inputs["node_b1"][l], np.float32).reshape(H, 1)
        shared[f"nb2_{l}"] = np.asarray(inputs["node_b2"][l], np.float32).reshape(H, 1)

    embow = np.asarray(inputs["emb_out_w"], np.float32)
    embob = np.asarray(inputs["emb_out_b"], np.float32)
    shared["embow"] = _to_bf(embow)

    # prot head: lhsT chunks [128,128] at col block (ki*4+mi)
    pw = np.asarray(inputs["prot_w"], np.float32)  # [PE, PH]
    nk, nm = cfg.pe // 128, cfg.ph // 128
    pw_sb = np.zeros((128, nk * nm * 128), np.float32)
    for ki in range(nk):
        for mi in range(nm):
            j = ki * nm + mi
            pw_sb[:, j * 128 : (j + 1) * 128] = pw[
                ki * 128 : (ki + 1) * 128, mi * 128 : (mi + 1) * 128
            ]
    shared["protw"] = _to_bf(pw_sb)
    shared["protb"] = np.asarray(inputs["prot_b"], np.float32).reshape(nm, 128).T.copy()

    # combined head: rows [drug(H); prot(PH)] -> comb_w1 [(H+PH), CH]
    cw = np.asarray(inputs["comb_w1"], np.float32).copy()
    cb1 = np.asarray(inputs["comb_b1"], np.float32).copy()
    # fold graph-mean (1/32) into drug rows, emb_out bias into comb bias
    cb1 = cb1 + embob @ cw[0:H]
    cw[0:H] *= 1.0 / cfg.graph
    nkc = (cfg.h + cfg.ph) // 128
    nmc = cfg.ch // 128
    cw_sb = np.zeros((128, nkc * nmc * 128), np.float32)
    for ki in range(nkc):
        for mi in range(nmc):
            j = ki * nmc + mi
            cw_sb[:, j * 128 : (j + 1) * 128] = cw[
                ki * 128 : (ki + 1) * 128, mi * 128 : (mi + 1) * 128
            ]
    shared["combw1"] = _to_bf(cw_sb)
    shared["combb1"] = cb1.reshape(nmc, 128).T.copy()
    w2c = np.asarray(inputs["comb_w2"], np.float32)  # [CH, 1]
    shared["combw2"] = _to_bf(w2c.reshape(cfg.ch // 128, 128).T.copy())
    shared["combb2"] = np.asarray(inputs["comb_b2"], np.float32).reshape(1, 1)

    # ---- per-core tensors
    bpc = cfg.blocks  # blocks per core
    for k in range(cfg.ncores):
        nb0 = k * cfg.nloc
        m = {}
        m.update(shared)
        hs = h0[nb0 : nb0 + cfg.nloc]
        ps = pos[nb0 : nb0 + cfg.nloc]
        tab0 = np.zeros((cfg.nloc, cfg.tw), ml_dtypes.bfloat16)
        tab0[:, 0:H] = _to_bf(hs)
        tab0[:, H : H + 3] = _to_bf(ps)
        m["tab0"] = tab0
        m["hT0"] = hs.T.copy()  # [128, nloc] f32
        pnm = np.zeros((128, 4 * bpc), np.float32)
        for b in range(bpc):
            pnm[:, 4 * b : 4 * b + 3] = ps[b * 128 : (b + 1) * 128]
        m["posnm0"] = pnm
        inv = np.zeros((128, bpc), np.float32)
        for b in range(bpc):
            inv[:, b] = invc_full[nb0 + b * 128 : nb0 + (b + 1) * 128]
        m["invc"] = inv

        colpad = np.zeros(cfg.slots, np.int32)
        rowrel = np.full(cfg.slots, -1.0, np.float32)
        attrT = np.zeros((9, cfg.slots), np.float32)
        for b in range(bpc):
            gb = k * bpc + b
            s, e = blk_starts[gb], blk_starts[gb + 1]
            n = e - s
            o = b * cfg.ep
            colpad[o : o + n] = col_s[s:e]
            rowrel[o : o + n] = (row_s[s:e] - nb0).astype(np.float32)
            attrT[1 : 1 + ED, o : o + n] = attr_s[s:e].T
        m["colidx"] = colpad.reshape(cfg.chunks, 128).T.copy()  # [128, chunks]
        m["rowrel"] = rowrel.reshape(cfg.chunks, 128).T.copy()
        m["attrT"] = _to_bf(attrT)
        m["protT"] = _to_bf(prot[k * cfg.gpc : (k + 1) * cfg.gpc].T.copy())
        in_maps.append(m)

    return in_maps


# ---------------------------------------------------------------- device build


def build(cfg: Cfg):
    H = cfg.h
    nc = bacc.Bacc(
        "TRN2",
        target_bir_lowering=False,
        debug=False,
        enable_asserts=False,
        num_devices=cfg.ncores,
    )

    # ---------------- external IO
    ext = {}

    def ein(name, shape, dt):
        ext[name] = nc.dram_tensor(name, list(shape), dt, kind="ExternalInput")
        return ext[name]

    for l in range(cfg.layers):
        ein(f"w1a_{l}", [H, H], BF16)
        ein(f"w1b_{l}", [H, H], BF16)
        ein(f"w1c_{l}", [9, H], BF16)
        ein(f"w2_{l}", [H, H], BF16)
        ein(f"attw_{l}", [H, 1], BF16)
        ein(f"cw1_{l}", [H, H], BF16)
        ein(f"cw2_{l}", [H, 1], BF16)
        ein(f"nw1h_{l}", [H, H], BF16)
        ein(f"nw1a_{l}", [H, H], BF16)
        ein(f"nw2_{l}", [H, H], BF16)
        ein(f"b1_{l}", [H, 1], F32)
        ein(f"b2_{l}", [H, 1], F32)
        ein(f"attb_{l}", [1, 1], F32)
        ein(f"cb1_{l}", [H, 1], F32)
        ein(f"nb1_{l}", [H, 1], F32)
        ein(f"nb2_{l}", [H, 1], F32)
    ein("embow", [H, H], BF16)
    nk, nm = cfg.pe // 128, cfg.ph // 128
    ein("protw", [128, nk * nm * 128], BF16)
    ein("protb", [128, nm], F32)
    nkc, nmc = (cfg.h + cfg.ph) // 128, cfg.ch // 128
    ein("combw1", [128, nkc * nmc * 128], BF16)
    ein("combb1", [128, nmc], F32)
    ein("combw2", [128, cfg.ch // 128], BF16)
    ein("combb2", [1, 1], F32)

    ein("tab0", [cfg.nloc, cfg.tw], BF16)
    ein("hT0", [128, cfg.nloc], F32)
    ein("posnm0", [128, 4 * cfg.blocks], F32)
    ein("invc", [128, cfg.blocks], F32)
    ein("colidx", [128, cfg.chunks], I32)
    ein("rowrel", [128, cfg.chunks], F32)
    ein("attrT", [9, cfg.slots], BF16)
    ein("protT", [cfg.pe, cfg.gpc], BF16)

    out_logits = nc.dram_tensor("logits", [1, cfg.gpc], F32, kind="ExternalOutput")

    rg = [list(range(cfg.ncores))]

    with tile.TileContext(nc) as tc:
        with (
            tc.tile_pool(name="dram", bufs=1, space="DRAM") as dpool,
            tc.tile_pool(name="const", bufs=1) as cpool,
            tc.tile_pool(name="big", bufs=1) as bigpool,
            tc.tile_pool(name="work", bufs=3) as wpool,
            tc.tile_pool(name="gath", bufs=8) as gpool,
            tc.tile_pool(name="small", bufs=8) as spool,
            tc.tile_pool(name="psum", bufs=3, space="PSUM") as pmm,
            tc.tile_pool(name="psum_tr", bufs=3, space="PSUM") as ptr,
            tc.tile_pool(name="psum_agg", bufs=2, space="PSUM") as pagg,
        ):
            # ------------ DRAM scratch
            from concourse.replica_groups import maybe_share_collective_output_space

            tab_space = maybe_share_collective_output_space("AllGather", rg)
            stage = dpool.tile([cfg.nloc, cfg.tw], BF16)
            tables = [
                dpool.tile([cfg.n, cfg.tw], BF16, addr_space=tab_space, name=f"table_{i}")
                for i in range(cfg.layers)
            ]
            onehotD = dpool.tile([cfg.blocks, cfg.ep, 128], BF16)

            # ------------ constants / resident state
            identity_bf = cpool.tile([128, 128], BF16)
            make_identity(nc, identity_bf[:, :])
            identity_f = cpool.tile([128, 128], F32)
            make_identity(nc, identity_f[:, :])
            ones_bf = cpool.tile([1, 128], BF16)
            nc.vector.memset(ones_bf[:, :], 1.0)
            iotaC_i = cpool.tile([128, 128], I32)
            nc.gpsimd.iota(iotaC_i[:, :], pattern=[[1, 128]], base=0, channel_multiplier=0)
            iotaC = cpool.tile([128, 128], F32)
            nc.vector.tensor_copy(iotaC[:, :], iotaC_i[:, :])

            hT = bigpool.tile([128, cfg.nloc], F32)
            nc.gpsimd.dma_start(hT[:, :], ext["hT0"][:, :])
            hTb = bigpool.tile([128, cfg.nloc], BF16)
            nc.gpsimd.dma_start(hTb[:, :], ext["hT0"][:, :])  # cast f32->bf16
            posnm = cpool.tile([128, 4 * cfg.blocks], F32)
            nc.gpsimd.dma_start(posnm[:, :], ext["posnm0"][:, :])
            posnb = cpool.tile([128, 4 * cfg.blocks], BF16)
            nc.vector.tensor_copy(posnb[:, :], posnm[:, :])
            invc = cpool.tile([128, cfg.blocks], F32)
            nc.gpsimd.dma_start(invc[:, :], ext["invc"][:, :])
            colidx = cpool.tile([128, cfg.chunks], I32)
            nc.gpsimd.dma_start(colidx[:, :], ext["colidx"][:, :])
            rowrel = cpool.tile([128, cfg.chunks], F32)
            nc.gpsimd.dma_start(rowrel[:, :], ext["rowrel"][:, :])

            # weights -> SBUF
            W = {}
            for l in range(cfg.layers):
                for nme, shp in [
                    (f"w1a_{l}", [H, H]), (f"w1b_{l}", [H, H]), (f"w1c_{l}", [9, H]),
                    (f"w2_{l}", [H, H]), (f"attw_{l}", [H, 1]), (f"cw1_{l}", [H, H]),
                    (f"cw2_{l}", [H, 1]), (f"nw1h_{l}", [H, H]), (f"nw1a_{l}", [H, H]),
                    (f"nw2_{l}", [H, H]),
                ]:
                    t = cpool.tile(shp, BF16, name=f"sb_{nme}")
                    nc.gpsimd.dma_start(t[:, :], ext[nme][:, :])
                    W[nme] = t
                for nme, shp in [
                    (f"b1_{l}", [H, 1]), (f"b2_{l}", [H, 1]), (f"attb_{l}", [1, 1]),
                    (f"cb1_{l}", [H, 1]), (f"nb1_{l}", [H, 1]), (f"nb2_{l}", [H, 1]),
                ]:
                    t = cpool.tile(shp, F32, name=f"sb_{nme}")
                    nc.gpsimd.dma_start(t[:, :], ext[nme][:, :])
                    W[nme] = t
            for nme, shp, dt in [
                ("embow", [H, H], BF16),
                ("protw", [128, nk * nm * 128], BF16), ("protb", [128, nm], F32),
                ("combw1", [128, nkc * nmc * 128], BF16), ("combb1", [128, nmc], F32),
                ("combw2", [128, cfg.ch // 128], BF16), ("combb2", [1, 1], F32),
            ]:
                t = cpool.tile(shp, dt, name=f"sb_{nme}")
                nc.gpsimd.dma_start(t[:, :], ext[nme][:, :])
                W[nme] = t

            # ------------ initial table broadcast
            nc.gpsimd.dma_start(stage[:, :], ext["tab0"][:, :])
            nc.gpsimd.collective_compute(
                "AllGather", ALU.bypass, replica_groups=rg,
                ins=[stage[:, :].opt()], outs=[tables[0][:, :].opt()],
            )

            # ------------ build onehot (edge-major) in DRAM
            for b in range(cfg.blocks):
                for c in range(cfg.cpb):
                    cg = b * cfg.cpb + c
                    rr = spool.tile([128, 1], F32, name="rr")
                    nc.vector.tensor_scalar_sub(rr[:, :], rowrel[:, cg : cg + 1], 128.0 * b)
                    oh = gpool.tile([128, 128], BF16, name="oh_build")
                    nc.vector.tensor_tensor(
                        oh[:, :], rr[:, :].to_broadcast([128, 128]), iotaC[:, :],
                        op=ALU.is_equal,
                    )
                    nc.gpsimd.dma_start(onehotD[b, c * 128 : (c + 1) * 128, :], oh[:, :])

            # ------------ layers
            for l in range(cfg.layers):
                last = l == cfg.layers - 1
                AW = 128 if last else 132
                tableA = tables[l]
                for b in range(cfg.blocks):
                    ohT = wpool.tile([128, cfg.ep], BF16, name="ohT")
                    nc.sync.dma_start(ohT[:, :], onehotD[b, :, :], transpose=True)

                    u_ps = pmm.tile([128, 128], F32, tag="mm", name="u_ps")
                    nc.tensor.matmul(
                        u_ps[:, :], lhsT=hTb[:, b * 128 : (b + 1) * 128],
                        rhs=W[f"w1a_{l}"][:, :], start=True, stop=True,
                    )
                    u_sb = wpool.tile([128, 128], BF16, name="u_sb")
                    nc.any.tensor_copy(u_sb[:, :], u_ps[:, :])

                    agg_ps = pagg.tile([128, AW], F32, tag="agg", name="agg_ps")

                    for g0 in range(0, cfg.cpb, 4):
                        nj = min(4, cfg.cpb - g0)
                        w = nj * 128
                        s0 = b * cfg.ep + g0 * 128  # slot offset in core arrays

                        gt = []
                        for j in range(nj):
                            cg = b * cfg.cpb + g0 + j
                            g = gpool.tile([128, cfg.tw], BF16, name="gath")
                            nc.gpsimd.indirect_dma_start(
                                out=g[:, :], out_offset=None,
                                in_=tableA[:, :],
                                in_offset=IndirectOffsetOnAxis(
                                    ap=colidx[:, cg : cg + 1], axis=0
                                ),
                            )
                            gt.append(g)

                        hcolT = wpool.tile([128, 512], BF16, name="hcolT")
                        for j in range(nj):
                            tp = ptr.tile([128, 128], BF16, tag="tr", name="tr_ps")
                            nc.tensor.transpose(tp[:, :], gt[j][:, 0:128], identity_bf[:, :])
                            nc.any.tensor_copy(hcolT[:, j * 128 : (j + 1) * 128], tp[:, :])

                        ar = wpool.tile([9, 512], BF16, name="ar")
                        nc.scalar.dma_start(ar[1:9, 0:w], ext["attrT"][1:9, s0 : s0 + w])

                        dn = []
                        for j in range(nj):
                            pr_ps = ptr.tile([128, 4], F32, tag="tr", name="pr_ps")
                            nc.tensor.matmul(
                                pr_ps[:, :],
                                lhsT=ohT[:, (g0 + j) * 128 : (g0 + j + 1) * 128],
                                rhs=posnb[:, 4 * b : 4 * b + 4], start=True, stop=True,
                            )
                            d_j = spool.tile([128, 4], F32, name="d_j")
                            nc.vector.tensor_tensor(
                                d_j[:, :], pr_ps[:, :], gt[j][:, H : H + 4], op=ALU.subtract
                            )
                            dsq = spool.tile([128, 4], F32, name="dsq")
                            nc.vector.tensor_tensor(dsq[:, :], d_j[:, :], d_j[:, :], op=ALU.mult)
                            rad = spool.tile([128, 1], F32, name="rad")
                            nc.vector.tensor_reduce(
                                rad[:, :], dsq[:, :], axis=mybir.AxisListType.X, op=ALU.add
                            )
                            rt_ps = ptr.tile([1, 128], F32, tag="tr", name="rt_ps")
                            nc.tensor.transpose(rt_ps[:, :], rad[:, :], identity_f[:, :])
                            nc.any.tensor_copy(ar[0:1, j * 128 : (j + 1) * 128], rt_ps[:, :])
                            if not last:
                                sq = spool.tile([128, 1], F32, name="sq")
                                nc.scalar.activation(sq[:, :], rad[:, :], AF.Sqrt)
                                nc.vector.tensor_scalar_add(sq[:, :], sq[:, :], EPS)
                                ri = spool.tile([128, 1], F32, name="ri")
                                nc.vector.reciprocal(ri[:, :], sq[:, :])
                                dnj = spool.tile([128, 4], F32, name="dnj")
                                nc.vector.tensor_tensor(
                                    dnj[:, :], d_j[:, :], ri[:, :].to_broadcast([128, 4]),
                                    op=ALU.mult,
                                )
                                dn.append(dnj)

                        e1 = pmm.tile([128, 512], F32, tag="mm", name="e1")
                        nc.tensor.matmul(
                            e1[:, 0:w], lhsT=u_sb[:, :],
                            rhs=ohT[:, g0 * 128 : g0 * 128 + w], start=True, stop=False,
                        )
                        nc.tensor.matmul(
                            e1[:, 0:w], lhsT=W[f"w1b_{l}"][:, :], rhs=hcolT[:, 0:w],
                            start=False, stop=False,
                        )
                        nc.tensor.matmul(
                            e1[:, 0:w], lhsT=W[f"w1c_{l}"][:, :], rhs=ar[:, 0:w],
                            start=False, stop=True,
                        )
                        m1 = wpool.tile([128, 512], BF16, name="m1")
                        nc.scalar.activation(
                            m1[:, 0:w], e1[:, 0:w], AF.Silu, bias=W[f"b1_{l}"][:, :]
                        )
                        e2 = pmm.tile([128, 512], F32, tag="mm", name="e2")
                        nc.tensor.matmul(
                            e2[:, 0:w], lhsT=W[f"w2_{l}"][:, :], rhs=m1[:, 0:w],
                            start=True, stop=True,
                        )
                        mm_ = wpool.tile([128, 512], BF16, name="mm_")
                        nc.scalar.activation(
                            mm_[:, 0:w], e2[:, 0:w], AF.Silu, bias=W[f"b2_{l}"][:, :]
                        )
                        att = pmm.tile([1, 512], F32, tag="mm", name="att")
                        nc.tensor.matmul(
                            att[:, 0:w], lhsT=W[f"attw_{l}"][:, :], rhs=mm_[:, 0:w],
                            start=True, stop=True,
                        )
                        gate = spool.tile([1, 512], BF16, name="gate")
                        nc.scalar.activation(
                            gate[:, 0:w], att[:, 0:w], AF.Sigmoid, bias=W[f"attb_{l}"][:, :]
                        )
                        gb = pmm.tile([128, 512], F32, tag="mm", name="gb")
                        nc.tensor.matmul(
                            gb[:, 0:w], lhsT=ones_bf[:, :], rhs=gate[:, 0:w],
                            start=True, stop=True,
                        )
                        mg = wpool.tile([128, 512], BF16, name="mg")
                        nc.vector.tensor_tensor(mg[:, 0:w], mm_[:, 0:w], gb[:, 0:w], op=ALU.mult)

                        if not last:
                            t_ps = pmm.tile([128, 512], F32, tag="mm", name="t_ps")
                            nc.tensor.matmul(
                                t_ps[:, 0:w], lhsT=W[f"cw1_{l}"][:, :], rhs=mg[:, 0:w],
                                start=True, stop=True,
                            )
                            tT = wpool.tile([128, 512], BF16, name="tT")
                            nc.scalar.activation(
                                tT[:, 0:w], t_ps[:, 0:w], AF.Silu, bias=W[f"cb1_{l}"][:, :]
                            )

                        oh4 = gpool.tile([128, nj, 128], BF16, name="oh4", tag="oh4")
                        src = onehotD[b, g0 * 128 : g0 * 128 + w, :]
                        nc.scalar.dma_start(
                            oh4[:, 0:nj, :], src.rearrange("(j p) n -> p j n", p=128)
                        )
                        for j in range(nj):
                            mt_ps = ptr.tile([128, 128], BF16, tag="tr", name="mt_ps")
                            nc.tensor.transpose(
                                mt_ps[:, :], mg[:, j * 128 : (j + 1) * 128], identity_bf[:, :]
                            )
                            srhs = gpool.tile([128, 132], BF16, name="srhs", tag="srhs")
                            nc.any.tensor_copy(srhs[:, 0:128], mt_ps[:, :])
                            if not last:
                                c2 = ptr.tile([128, 4], F32, tag="tr", name="c2")
                                nc.tensor.matmul(
                                    c2[:, 0:1], lhsT=tT[:, j * 128 : (j + 1) * 128],
                                    rhs=W[f"cw2_{l}"][:, :], start=True, stop=True,
                                )
                                th = spool.tile([128, 1], F32, name="th")
                                nc.scalar.activation(th[:, :], c2[:, 0:1], AF.Tanh)
                                nc.vector.tensor_tensor(
                                    srhs[:, 128:132], dn[j][:, :],
                                    th[:, :].to_broadcast([128, 4]), op=ALU.mult,
                                )
                            nc.tensor.matmul(
                                agg_ps[:, :], lhsT=oh4[:, j, :], rhs=srhs[:, 0:AW],
                                start=(g0 + j == 0), stop=(g0 + j == cfg.cpb - 1),
                            )

                    # ---- node update for block b
                    blk = slice(b * 128, (b + 1) * 128)
                    if not last:
                        pd = spool.tile([128, 3], F32, name="pd")
                        nc.vector.tensor_tensor(
                            pd[:, :], agg_ps[:, 128:131],
                            invc[:, b : b + 1].to_broadcast([128, 3]), op=ALU.mult,
                        )
                        nc.vector.tensor_tensor(
                            posnm[:, 4 * b : 4 * b + 3], posnm[:, 4 * b : 4 * b + 3],
                            pd[:, :], op=ALU.add,
                        )
                        nc.vector.tensor_copy(
                            posnb[:, 4 * b : 4 * b + 4], posnm[:, 4 * b : 4 * b + 4]
                        )

                    agg_nm = wpool.tile([128, 128], BF16, name="agg_nm")
                    nc.any.tensor_copy(agg_nm[:, :], agg_ps[:, 0:128])
                    at_ps = ptr.tile([128, 128], BF16, tag="tr", name="at_ps")
                    nc.tensor.transpose(at_ps[:, :], agg_nm[:, :], identity_bf[:, :])
                    aggT = wpool.tile([128, 128], BF16, name="aggT")
                    nc.any.tensor_copy(aggT[:, :], at_ps[:, :])

                    nh_ps = pmm.tile([128, 128], F32, tag="mm", name="nh_ps")
                    nc.tensor.matmul(
                        nh_ps[:, :], lhsT=W[f"nw1h_{l}"][:, :], rhs=hTb[:, blk],
                        start=True, stop=False,
                    )
                    nc.tensor.matmul(
                        nh_ps[:, :], lhsT=W[f"nw1a_{l}"][:, :], rhs=aggT[:, :],
                        start=False, stop=True,
                    )
                    nh = wpool.tile([128, 128], BF16, name="nh")
                    nc.scalar.activation(nh[:, :], nh_ps[:, :], AF.Silu, bias=W[f"nb1_{l}"][:, :])
                    h2_ps = pmm.tile([128, 128], F32, tag="mm", name="h2_ps")
                    nc.tensor.matmul(
                        h2_ps[:, :], lhsT=W[f"nw2_{l}"][:, :], rhs=nh[:, :],
                        start=True, stop=True,
                    )
                    hd = wpool.tile([128, 128], F32, name="hd")
                    nc.scalar.activation(hd[:, :], h2_ps[:, :], AF.Identity, bias=W[f"nb2_{l}"][:, :])
                    nc.vector.tensor_tensor(hT[:, blk], hT[:, blk], hd[:, :], op=ALU.add)
                    nc.any.tensor_copy(hTb[:, blk], hT[:, blk])

                    if not last:
                        hn_ps = ptr.tile([128, 128], BF16, tag="tr", name="hn_ps")
                        nc.tensor.transpose(hn_ps[:, :], hTb[:, blk], identity_bf[:, :])
                        hnm = wpool.tile([128, 128], BF16, name="hnm")
                        nc.any.tensor_copy(hnm[:, :], hn_ps[:, :])
                        nc.gpsimd.dma_start(stage[blk, 0:128], hnm[:, :])
                        nc.gpsimd.dma_start(
                            stage[blk, H : H + 4],
                            posnb[:, 4 * b : 4 * b + 4],
                        )

                if not last:
                    nc.gpsimd.collective_compute(
                        "AllGather", ALU.bypass, replica_groups=rg,
                        ins=[stage[:, :].opt()], outs=[tables[l + 1][:, :].opt()],
                    )

            # ------------ heads
            GPC, G = cfg.gpc, cfg.graph
            drug = bigpool.tile([128, GPC], F32, name="drug")
            hch = min(512, cfg.nloc)  # nodes per head chunk
            npg = hch // G  # graphs per head chunk
            for ci in range(cfg.nloc // hch):
                ho = pmm.tile([128, 512], F32, tag="mm", name="ho")
                nc.tensor.matmul(
                    ho[:, 0:hch], lhsT=W["embow"][:, :],
                    rhs=hTb[:, ci * hch : (ci + 1) * hch],
                    start=True, stop=True,
                )
                nc.vector.tensor_reduce(
                    drug[:, ci * npg : (ci + 1) * npg],
                    ho[:, 0:hch].rearrange("p (g t) -> p g t", t=G),
                    axis=mybir.AxisListType.X, op=ALU.add,
                )
            drugb = bigpool.tile([128, GPC], BF16, name="drugb")
            nc.vector.tensor_copy(drugb[:, :], drug[:, :])

            protc = []
            for mi in range(nm):
                pp = pmm.tile([128, GPC], F32, tag="mm", name="pp")
                for ki in range(nk):
                    pt = wpool.tile([128, GPC], BF16, name="pt")
                    nc.scalar.dma_start(pt[:, :], ext["protT"][ki * 128 : (ki + 1) * 128, :])
                    nc.tensor.matmul(
                        pp[:, :], lhsT=W["protw"][:, (ki * nm + mi) * 128 : (ki * nm + mi + 1) * 128],
                        rhs=pt[:, :], start=(ki == 0), stop=(ki == nk - 1),
                    )
                pc = bigpool.tile([128, GPC], BF16, name=f"protc_{mi}")
                nc.scalar.activation(pc[:, :], pp[:, :], AF.Relu, bias=W["protb"][:, mi : mi + 1])
                protc.append(pc)

            cc = []
            for mi in range(nmc):
                cp = pmm.tile([128, GPC], F32, tag="mm", name="cp")
                for kj in range(nkc):
                    rhs = drugb if kj == 0 else protc[kj - 1]
                    nc.tensor.matmul(
                        cp[:, :], lhsT=W["combw1"][:, (kj * nmc + mi) * 128 : (kj * nmc + mi + 1) * 128],
                        rhs=rhs[:, :], start=(kj == 0), stop=(kj == nkc - 1),
                    )
                ct = bigpool.tile([128, GPC], BF16, name=f"cc_{mi}")
                nc.scalar.activation(ct[:, :], cp[:, :], AF.Relu, bias=W["combb1"][:, mi : mi + 1])
                cc.append(ct)

            lg = pmm.tile([1, GPC], F32, tag="mm", name="lg")
            for kj in range(cfg.ch // 128):
                nc.tensor.matmul(
                    lg[:, :], lhsT=W["combw2"][:, kj : kj + 1], rhs=cc[kj][:, :],
                    start=(kj == 0), stop=(kj == cfg.ch // 128 - 1),
                )
            lgs = spool.tile([1, 512], F32, name="lgs")
            nc.scalar.activation(lgs[:, 0:GPC], lg[:, :], AF.Identity, bias=W["combb2"][:, :])
            nc.gpsimd.dma_start(out_logits[:, :], lgs[:, 0:GPC])

    nc.compile()
    return nc


# ---------------------------------------------------------------- entry point

_CACHE = {}


def _get_nc(cfg: Cfg):
    key = (cfg.ncores, cfg.nloc, cfg.ep, cfg.layers)
    if key not in _CACHE:
        _CACHE[key] = build(cfg)
    return _CACHE[key]


def kernel(**inputs):
    cfg = Cfg()
    in_maps = prep_inputs(cfg, inputs)
    nc = _get_nc(cfg)
    res = run_bass_kernel_spmd(nc, in_maps, core_ids=list(range(cfg.ncores)))
    logits = np.concatenate(
        [res.results[k]["logits"].reshape(-1) for k in range(cfg.ncores)]
    ).astype(np.float32)
    B = cfg.ncores * cfg.gpc
    attn = np.zeros((B, 1, 1), np.float32)
    return logits, attn
